# revision 8
# baseline (speedup 1.0000x reference)
"""DeltaRuleGated Trainium2 kernel.

Recurrence per (b,h) pair over T time steps, state M[128,128]:
    M_t = M_{t-1} * max(f_t (x) f_t, 0.8) + (k_t*g_t) (x) (v_t*g_t)
    o_t = q_t^T M_t
(upper clip at 1.0 is a no-op: f in [0,1) so f_d*f_e < 1)

Sharding: 32 (b,h) pairs -> 8 cores x 4 pairs, no cross-core comm.

Per-core design (per time step, 4 pairs batched at [128, 512]):
  - PE: outer products via K=3 bf16 hi/lo split matmuls (exact to ~1.5e-5):
        ff = fhi(x)fhi + fhi(x)flo + flo(x)fhi  -> PSUM bankF
        dd = uhi(x)whi + uhi(x)wlo + ulo(x)whi  -> PSUM bankD
    All pairs use PE row-group 0 for ff and row-group 1 for uw.
    (A PSUM bank must only ever be written from ONE tile_position, and a
    stationary operand's base partition must equal tile_position[0].)
  - DVE A: M' = max(bankF, 0.8) * M      (one fused scalar_tensor_tensor)
  - ACT:   Dsb = copy(bankD)             (PSUM->SBUF evacuation)
  - DVE B: M = M' + Dsb
  - PE matvec: masked-Q trick. Q_SINGLE_j = [128,C] tile, zero except
    column j = q_{t0+j}. C accumulating matmuls into PSUM bankO give
    o_{t0+j} in row j with a single [C,512] evacuation per C steps.
"""

import numpy as np

import concourse.bass as bass
import concourse.bacc as bacc
import concourse.tile as tile
from concourse import mybir
from concourse.bass_utils import run_bass_kernel_spmd

B, T, H, D = 4, 2048, 8, 128
N_CORES = 8
NP = (B * H) // N_CORES  # pairs per core = 4
C = 16                   # time steps per chunk (= output group size)
F32 = mybir.dt.float32
BF16 = mybir.dt.bfloat16
AOP = mybir.AluOpType
AF = mybir.ActivationFunctionType
PSUM = bass.MemorySpace.PSUM


def build(t_run=T):
    nch = t_run // C
    nc = bacc.Bacc(None, target_bir_lowering=False)

    dq = nc.dram_tensor("q", [NP, t_run, D], F32, kind="ExternalInput")
    dk = nc.dram_tensor("k", [NP, t_run, D], F32, kind="ExternalInput")
    dv = nc.dram_tensor("v", [NP, t_run, D], F32, kind="ExternalInput")
    df = nc.dram_tensor("f", [NP, t_run, D], F32, kind="ExternalInput")
    dg = nc.dram_tensor("g", [NP, t_run, D], F32, kind="ExternalInput")
    dident = nc.dram_tensor("ident", [D, D], F32, kind="ExternalInput")
    dout = nc.dram_tensor("out", [NP, t_run, D], F32, kind="ExternalOutput")

    with tile.TileContext(nc) as tc:
        with (
            tc.tile_pool(name="singles", bufs=1) as singles,
            tc.tile_pool(name="stage", bufs=2) as stage,
            tc.tile_pool(name="prep", bufs=2) as prep,
            tc.tile_pool(name="wload", bufs=2) as wload,
            tc.tile_pool(name="state", bufs=2) as statep,
            tc.tile_pool(name="step", bufs=3) as stepp,
            tc.tile_pool(name="outp", bufs=2) as outp,
            tc.tile_pool(name="psF", bufs=1, space=PSUM) as psF,
            tc.tile_pool(name="psD", bufs=2, space=PSUM) as psD,
            tc.tile_pool(name="psO", bufs=2, space=PSUM) as psO,
            tc.tile_pool(name="psT", bufs=1, space=PSUM) as psT,
        ):
            identS = singles.tile([D, D], F32)
            nc.sync.dma_start(out=identS[:, :], in_=dident[:, :])

            # Q_SINGLE regions: per pair [128, C*(C+1)]; tile_j = flat cols
            # [C*j, C*j+C); scattered q column lands at flat col (C+1)*j,
            # i.e. local col j of tile_j. Other cols stay zero forever.
            qsr = [
                singles.tile([D, C * (C + 1)], F32, name=f"qsr{p}", tag=f"qsr{p}")
                for p in range(NP)
            ]
            for p in range(NP):
                nc.gpsimd.memset(qsr[p][:, :], 0.0)

            # initial state M = 0
            m_prev = statep.tile([D, NP * D], F32, tag="M")
            nc.gpsimd.memset(m_prev[:, :], 0.0)

            for ch in range(nch):
                t0 = ch * C
                # ---- input staging: [C, NP, D] f32, iteration (t, p, d)
                fS = stage.tile([C, NP, D], F32, tag="fS")
                kS = stage.tile([C, NP, D], F32, tag="kS")
                vS = stage.tile([C, NP, D], F32, tag="vS")
                gS = stage.tile([C, NP, D], F32, tag="gS")
                qS = stage.tile([C, NP, D], F32, tag="qS")
                for dst, src in ((fS, df), (kS, dk), (vS, dv), (gS, dg), (qS, dq)):
                    nc.sync.dma_start(
                        out=dst[:, :, :],
                        in_=src[:, t0 : t0 + C, :].rearrange("p t d -> t p d"),
                    )

                # ---- gating precompute + bf16 hi/lo splits
                uF = prep.tile([C, NP, D], F32, tag="uF")
                wF = prep.tile([C, NP, D], F32, tag="wF")
                nc.gpsimd.tensor_mul(uF[:, :, :], kS[:, :, :], gS[:, :, :])
                nc.gpsimd.tensor_mul(wF[:, :, :], vS[:, :, :], gS[:, :, :])

                def hilo(x, tagbase):
                    hi = prep.tile([C, NP, D], BF16, name=tagbase + "hi", tag=tagbase + "hi")
                    lo = prep.tile([C, NP, D], BF16, name=tagbase + "lo", tag=tagbase + "lo")
                    nc.scalar.activation(hi[:, :, :], x[:, :, :], AF.Copy)
                    nc.gpsimd.tensor_sub(lo[:, :, :], x[:, :, :], hi[:, :, :])
                    return hi, lo

                fhi, flo = hilo(fS, "f")
                uhi, ulo = hilo(uF, "u")
                whi, wlo = hilo(wF, "w")

                # ---- stationary/stream weight tiles [35, NP*C*D] bf16
                # rows 0-2: f/u hi-lo pattern, rows 32-34: u/w. Pair p at
                # free offset p*C*D.
                stat = wload.tile([35, NP * C * D], BF16, tag="stat")
                strm = wload.tile([35, NP * C * D], BF16, tag="strm")

                for p in range(NP):
                    fo = p * C * D
                    # stationary rows: f_hi, f_hi, f_lo / u_hi, u_hi, u_lo
                    for r, srcT in (
                        (0, fhi), (1, fhi), (2, flo),
                        (32, uhi), (33, uhi), (34, ulo),
                    ):
                        nc.sync.dma_start(
                            out=stat[r : r + 1, fo : fo + C * D], in_=srcT[:, p, :]
                        )
                    # stream rows: f_hi, f_lo, f_hi / w_hi, w_lo, w_hi
                    for r, srcT in (
                        (0, fhi), (1, flo), (2, fhi),
                        (32, whi), (33, wlo), (34, whi),
                    ):
                        nc.sync.dma_start(
                            out=strm[r : r + 1, fo : fo + C * D], in_=srcT[:, p, :]
                        )

                # ---- q transpose -> scatter into Q_SINGLE regions
                for p in range(NP):
                    bankT = psT.tile([D, C], F32, tag="bankT")
                    nc.tensor.transpose(bankT[:, :], qS[:, p, :], identS[0:C, 0:C])
                    qT = stepp.tile([D, C, 1], F32, tag="qT")
                    nc.scalar.activation(
                        qT[:, :, 0:1],
                        bankT[:, :].rearrange("a (j o) -> a j o", o=1),
                        AF.Copy,
                    )
                    qv = qsr[p].rearrange("a (j c) -> a j c", c=C + 1)
                    nc.gpsimd.tensor_copy(qv[:, :, 0:1], qT[:, :, 0:1])

                oS = outp.tile([C, NP, D], F32, tag="oS")
                bankO = psO.tile([C, NP * D], F32, tag="bankO")

                # ---- the sequential scan over this chunk's steps
                for j in range(C):
                    bankF = psF.tile([D, NP * D], F32, tag="bankF")
                    bankD = psD.tile([D, NP * D], F32, tag="bankD")
                    for p in range(NP):
                        js = slice(p * C * D + j * D, p * C * D + (j + 1) * D)
                        ps = slice(p * D, (p + 1) * D)
                        nc.tensor.matmul(
                            bankF[:, ps], stat[0:3, js], strm[0:3, js],
                            start=True, stop=True, tile_position=(0, 0),
                        )
                        nc.tensor.matmul(
                            bankD[:, ps], stat[32:35, js], strm[32:35, js],
                            start=True, stop=True, tile_position=(32, 0),
                        )

                    dsb = stepp.tile([D, NP * D], F32, tag="dsb")
                    nc.scalar.activation(dsb[:, :], bankD[:, :], AF.Copy)

                    mp = stepp.tile([D, NP * D], F32, tag="mp")
                    nc.vector.scalar_tensor_tensor(
                        out=mp[:, :], in0=bankF[:, :], scalar=0.8, in1=m_prev[:, :],
                        op0=AOP.max, op1=AOP.mult,
                    )
                    m_new = statep.tile([D, NP * D], F32, tag="M")
                    nc.vector.tensor_add(m_new[:, :], mp[:, :], dsb[:, :])

                    for p in range(NP):
                        ps = slice(p * D, (p + 1) * D)
                        nc.tensor.matmul(
                            bankO[:, ps],
                            qsr[p][:, j * C : (j + 1) * C],
                            m_new[:, ps],
                            start=(j == 0 and p == 0),
                            stop=(j == C - 1 and p == NP - 1),
                            tile_position=(0, 0),
                        )
                    m_prev = m_new

                nc.scalar.activation(
                    oS[:, :, :],
                    bankO[:, :].rearrange("t (p d) -> t p d", d=D),
                    AF.Copy,
                )
                nc.sync.dma_start(
                    out=dout[:, t0 : t0 + C, :].rearrange("p t d -> t p d"),
                    in_=oS[:, :, :],
                )

    nc.compile()
    return nc


_CACHE = {}


def _get_nc(t_run):
    if t_run not in _CACHE:
        _CACHE[t_run] = build(t_run)
    return _CACHE[t_run]


def kernel(q, k, v, f_gate, g_gate):
    t_run = q.shape[1]
    nc = _get_nc(t_run)

    def shard(x):
        # [B, T, H, D] -> [B*H, T, D] -> per-core [NP, T, D]
        xt = np.ascontiguousarray(
            np.transpose(np.asarray(x, dtype=np.float32), (0, 2, 1, 3))
        ).reshape(B * H, t_run, D)
        return [np.ascontiguousarray(xt[c * NP : (c + 1) * NP]) for c in range(N_CORES)]

    qs, ks, vs, fs, gs = (shard(x) for x in (q, k, v, f_gate, g_gate))
    ident = np.eye(D, dtype=np.float32)
    in_maps = [
        {"q": qs[c], "k": ks[c], "v": vs[c], "f": fs[c], "g": gs[c], "ident": ident}
        for c in range(N_CORES)
    ]
    res = run_bass_kernel_spmd(nc, in_maps, core_ids=list(range(N_CORES)))
    full = np.concatenate([res.results[c]["out"] for c in range(N_CORES)], axis=0)
    # [B*H, T, D] -> [B, T, H, D]
    return np.ascontiguousarray(
        np.transpose(full.reshape(B, H, t_run, D), (0, 2, 1, 3))
    )
